# revision 17
# baseline (speedup 1.0000x reference)
"""Trainium2 Bass kernel for nn_Encoder_68324339745355 (sLSTM encoder).

Math (per reference):
    x  = obs @ W_sp.T + b_sp + img1 + img2          # [T,B,E]
    g  = x @ W_ih.T + h @ W_hh.T + b_g              # per step, [B,4H]
    i=exp(it) f=sig(ft) z=tanh(zt) o=sig(ot)
    c = f*c + i*z ; n = f*n + i ; h = o*c/n
    outputs: (out [T,B,H], h[None], c[None], n[None])

Distribution: pure data-parallel, batch 16384 -> 8 cores x 2048.

Per-core layout ("stacked halves", feature-major):
  The 2048-batch shard splits into two 1024-halves. Every on-chip tensor is
  [128 partitions x 1024], partition = 64*half + feature. Feature-major means
  the recurrent matmul needs no transposes anywhere: PE contracts over
  partitions, gates come out stacked the same way, and ACT/DVE always run
  full 128-partition ops.

  XH0 [128,1024]: rows 0:64 = h (half0), rows 64:128 = x (half0)
  XH1 [128,1024]: rows 0:64 = x (half1), rows 64:128 = h (half1)
  (flip forced by partition-preservation of DVE ops: h emerges on the same
   partitions as its gate psum rows)

x = img1 + img2 + obs@W_sp.T is folded on the host during input packing (the
spatial-embedding GEMM is 0.1% of model FLOPs; folding it into the packed
input stream halves input DMA and removes all x-merge work on device), so x
enters each step's XH tiles by straight DMA.

Gates for (gate-chunk g, half): one K=128 matmul, lhsT = [W_hh.T ; W_ih.T]
stacked to match XH rows. The two halves write psum partitions 0:64 / 64:128
(different PE column groups) and are emitted adjacently so the two streams
run concurrently on the PE.

Sigmoid via tanh (only the exp_and_others ACT table set is ever loaded):
  f = 0.5*tanh(ft/2)+0.5 via a 4x-mode tensor_scalar, o likewise.
  1/n via the 1-instruction approximate-reciprocal custom DVE op (bf16-safe).

dtypes: bf16 on-chip except f32 PSUM accumulation and the f32 ACT bias.
"""

import numpy as np
import ml_dtypes

T, B, H, E, DIN = 32, 16384, 64, 64, 4
NCORES = 8
BS = B // NCORES          # 2048 batch per core
SB = BS // 2              # 1024 free dim (two halves stacked on partitions)
CH = 512                  # free-dim chunk (psum bank / moving-operand limit)
NCH = SB // CH

bf16 = ml_dtypes.bfloat16

_CACHE = {}


def _get_crf_op():
    """Register (once) a fused custom DVE op: out = Src1 * approx(1/Src0).

    Same bitwise-NOT seed as RECIPROCAL_APPROX_FAST but a single Newton
    step (max rel err ~0.4%, at bf16 noise level), freeing pipeline stages
    to fuse the multiply by c' — replaces reciprocal + tensor_tensor with
    one DVE instruction."""
    if "crf" in _CACHE:
        return _CACHE["crf"]
    import concourse.dve_ops as dve_ops
    from concourse.dve_spec import Spec, Src0, Src1, C0, C1, Bin, AluOp, lower
    from concourse.dve_uop import DveOpSpec
    import numpy as np_

    _not = Bin(AluOp.BITWISE_NOT, Src0, Src0)
    _y0 = _not * C0
    _y1 = _y0 * (C1 - Src0 * _y0)

    def _ref(in0, in1, s0, s1, imm2):
        not_x = (~in0.astype(np_.float32).view(np_.int32)).view(np_.float32)
        y0 = not_x * s0
        y1 = y0 * (s1 - in0 * y0)
        return (y1 * in1).astype(np_.float32)

    spec = Spec(body=_y1 * Src1, reference=_ref)
    name = "RECIP1_MUL_ANT"
    row = dve_ops._CUSTOM_DVE_ROW_BASE + len(dve_ops.OPS)
    assert row < 0x20
    shas = {}
    for ver in ("v3",):
        tmp = DveOpSpec(name=name, opcode=row, uops=lower(spec, ver=ver), rd1_en=True)
        shas[ver] = tmp.sha(ver)
    op = dve_ops.DveOp(name, spec, subdim=False, uops_sha=shas)
    dve_ops.OPS.append(op)
    dve_ops._SUB_OPCODE_FOR_NAME[name] = row
    _CACHE["crf"] = op
    return op


def _build(repeat=1):
    import concourse.bass as bass
    import concourse.bacc as bacc
    import concourse.tile as tile
    import concourse.mybir as mybir
    from concourse.dve_ops import RECIP_APPROX_FAST_CONSTS, RECIPROCAL_APPROX_FAST

    dt = mybir.dt
    AF = mybir.ActivationFunctionType
    ALU = mybir.AluOpType

    nc = bacc.Bacc("TRN2", target_bir_lowering=False, debug=False,
                   num_devices=NCORES)

    crf = _get_crf_op()
    imx = nc.dram_tensor("imx", [T, 128, SB], dt.bfloat16, kind="ExternalInput").ap()
    wh0 = nc.dram_tensor("wh0", [128, 256], dt.bfloat16, kind="ExternalInput").ap()
    wh1 = nc.dram_tensor("wh1", [128, 256], dt.bfloat16, kind="ExternalInput").ap()
    bia = nc.dram_tensor("bia", [128, 4], dt.float32, kind="ExternalInput").ap()
    out2h = nc.dram_tensor("out2h", [T, 128, SB], dt.bfloat16, kind="ExternalOutput").ap()
    cfin = nc.dram_tensor("cfin", [128, SB], dt.bfloat16, kind="ExternalOutput").ap()
    nfin = nc.dram_tensor("nfin", [128, SB], dt.bfloat16, kind="ExternalOutput").ap()

    rc = RECIP_APPROX_FAST_CONSTS

    with tile.TileContext(nc) as tc:
        with (
            tc.tile_pool(name="pw", bufs=1) as pw,
            tc.tile_pool(name="pxh", bufs=3) as pxh,
            tc.tile_pool(name="pst", bufs=2) as pst,
            tc.tile_pool(name="pg", bufs=2) as pg,
            tc.tile_pool(name="pt", bufs=2) as pt,
            tc.tile_pool(name="pps", bufs=2, space="PSUM") as pps,
        ):
            WH0 = pw.tile([128, 256], dt.bfloat16, name="WH0")
            WH1 = pw.tile([128, 256], dt.bfloat16, name="WH1")
            BIA = pw.tile([128, 4], dt.float32, name="BIA")
            nc.sync.dma_start(out=WH0[:], in_=wh0[:])
            nc.sync.dma_start(out=WH1[:], in_=wh1[:])
            nc.sync.dma_start(out=BIA[:], in_=bia[:])

            def new_xh(t):
                x0 = pxh.tile([128, SB], dt.bfloat16, name=f"XH0_{t}", tag="XH0")
                x1 = pxh.tile([128, SB], dt.bfloat16, name=f"XH1_{t}", tag="XH1")
                return x0, x1

            def load_x(t, x0, x1):
                # imx rows 0:64 = half1 x, rows 64:128 = half0 x
                nc.sync.dma_start(out=x0[64:128, :], in_=imx[t, 64:128, :])
                nc.sync.dma_start(out=x1[0:64, :], in_=imx[t, 0:64, :])

            XH0c, XH1c = new_xh(-1)
            nc.vector.memset(XH0c[0:64, :], 0.0)
            nc.vector.memset(XH1c[64:128, :], 0.0)
            load_x(0, XH0c, XH1c)
            Cc = pst.tile([128, SB], dt.bfloat16, name="C_init", tag="C")
            Nc = pst.tile([128, SB], dt.bfloat16, name="N_init", tag="N")
            nc.vector.memset(Cc[:], 0.0)
            nc.vector.memset(Nc[:], 0.0)

            for t in range(T * repeat):
                t = t % T
                XH0n, XH1n = new_xh(t)
                Cn = pst.tile([128, SB], dt.bfloat16, name=f"C_{t}", tag="C")
                Nn = pst.tile([128, SB], dt.bfloat16, name=f"N_{t}", tag="N")
                Hn = pst.tile([128, SB], dt.bfloat16, name=f"H_{t}", tag="Hh")
                if t + 1 < T:
                    load_x(t + 1, XH0n, XH1n)

                for ch in range(NCH):
                    cs = slice(ch * CH, ch * CH + CH)
                    PS = [pps.tile([128, CH], dt.float32, name=f"PS{g}_{t}_{ch}",
                                   tag=f"PS{g}") for g in range(4)]
                    for g in range(4):
                        gsl = slice(64 * g, 64 * g + 64)
                        nc.tensor.matmul(PS[g][0:64, :], WH0[:, gsl], XH0c[:, cs],
                                         start=True, stop=True)
                        nc.tensor.matmul(PS[g][64:128, :], WH1[:, gsl], XH1c[:, cs],
                                         start=True, stop=True)

                    I = pg.tile([128, CH], dt.bfloat16, name=f"I_{t}_{ch}", tag="I")
                    Z = pg.tile([128, CH], dt.bfloat16, name=f"Z_{t}_{ch}", tag="Z")
                    U = pg.tile([128, CH], dt.bfloat16, name=f"U_{t}_{ch}", tag="U")
                    V = pg.tile([128, CH], dt.bfloat16, name=f"V_{t}_{ch}", tag="V")
                    nc.scalar.activation(I[:], PS[0][:], AF.Exp, bias=BIA[:, 0:1])
                    nc.scalar.activation(Z[:], PS[2][:], AF.Tanh, bias=BIA[:, 2:3])
                    nc.scalar.activation(U[:], PS[1][:], AF.Tanh, bias=BIA[:, 1:2], scale=0.5)
                    nc.scalar.activation(V[:], PS[3][:], AF.Tanh, bias=BIA[:, 3:4], scale=0.5)

                    F = pt.tile([128, CH], dt.bfloat16, name=f"F_{t}_{ch}", tag="F")
                    O = pt.tile([128, CH], dt.bfloat16, name=f"O_{t}_{ch}", tag="O")
                    IZ = pt.tile([128, CH], dt.bfloat16, name=f"IZ_{t}_{ch}", tag="IZ")
                    FC = pt.tile([128, CH], dt.bfloat16, name=f"FC_{t}_{ch}", tag="FC")
                    FN = pt.tile([128, CH], dt.bfloat16, name=f"FN_{t}_{ch}", tag="FN")
                    CR = pt.tile([128, CH], dt.bfloat16, name=f"CR_{t}_{ch}", tag="CR")

                    nc.vector.tensor_scalar(F[:], U[:], 0.5, 0.5, ALU.mult, ALU.add)
                    nc.vector.tensor_scalar(O[:], V[:], 0.5, 0.5, ALU.mult, ALU.add)
                    nc.vector.tensor_tensor(IZ[:], I[:], Z[:], ALU.mult)
                    nc.vector.tensor_tensor(FC[:], F[:], Cc[:, cs], ALU.mult)
                    nc.vector.tensor_tensor(Cn[:, cs], FC[:], IZ[:], ALU.add)
                    nc.vector.tensor_tensor(FN[:], F[:], Nc[:, cs], ALU.mult)
                    nc.vector.tensor_tensor(Nn[:, cs], FN[:], I[:], ALU.add)
                    # cr = c' * approx(1/n') in one fused DVE op
                    nc.vector._custom_dve(crf, out=CR[:], in0=Nn[:, cs], in1=Cn[:, cs],
                                          s0=rc["s0"], s1=rc["s1"])
                    nc.vector.tensor_tensor(Hn[:, cs], O[:], CR[:], ALU.mult)
                    # redistribute h into next step's XH tiles via the idle
                    # gpsimd DMA queue (keeps it off both DVE and the SP queue)
                    nc.gpsimd.dma_start(out=XH0n[0:64, cs], in_=Hn[0:64, cs])
                    nc.gpsimd.dma_start(out=XH1n[64:128, cs], in_=Hn[64:128, cs])

                nc.sync.dma_start(out=out2h[t], in_=Hn[:])

                XH0c, XH1c = XH0n, XH1n
                Cc, Nc = Cn, Nn

            nc.sync.dma_start(out=cfin[:], in_=Cc[:])
            nc.sync.dma_start(out=nfin[:], in_=Nc[:])

    nc.compile()
    return nc


def _get_nc(repeat=1):
    key = f"nc{repeat}"
    if key not in _CACHE:
        _CACHE[key] = _build(repeat)
    return _CACHE[key]


def _pack_feature_major(arr, core):
    # arr [T, B, F] -> [T, 2F, SB] with rows 0:F = half1.T, rows F:2F = half0.T
    F = arr.shape[2]
    a = arr[:, core * BS:(core + 1) * BS, :].reshape(T, 2, SB, F)
    a = a.transpose(0, 1, 3, 2)  # [T, 2, F, SB]
    return np.concatenate([a[:, 1], a[:, 0]], axis=1)


def make_in_maps(obs_traj, img_embed_input, img_embed_input_merge, W_sp, b_sp,
                 W_ih, W_hh, b_g):
    obs_traj = np.asarray(obs_traj, dtype=np.float32)
    img1 = np.asarray(img_embed_input, dtype=np.float32)
    img2 = np.asarray(img_embed_input_merge, dtype=np.float32)
    W_sp = np.asarray(W_sp, dtype=np.float32)
    b_sp = np.asarray(b_sp, dtype=np.float32)
    W_ih = np.asarray(W_ih, dtype=np.float32)
    W_hh = np.asarray(W_hh, dtype=np.float32)
    b_g = np.asarray(b_g, dtype=np.float32)

    whh_t = W_hh.T.astype(bf16).astype(np.float32)   # [H, 4H]
    wih_t = W_ih.T.astype(bf16).astype(np.float32)   # [E, 4H]
    wh0 = np.concatenate([whh_t, wih_t], axis=0).astype(bf16)  # XH0 rows: [h ; x]
    wh1 = np.concatenate([wih_t, whh_t], axis=0).astype(bf16)  # XH1 rows: [x ; h]
    bias = b_g + W_ih @ b_sp
    bi = np.concatenate([bias[0:64], bias[0:64]])
    bf_ = np.concatenate([bias[64:128], bias[64:128]]) * 0.5
    bz = np.concatenate([bias[128:192], bias[128:192]])
    bo = np.concatenate([bias[192:256], bias[192:256]]) * 0.5
    bia = np.stack([bi, bf_, bz, bo], axis=1).astype(np.float32)

    # fold the spatial embedding into the packed x stream
    x = img1 + img2 + np.einsum("tbd,ed->tbe", obs_traj, W_sp)

    in_maps = []
    for core in range(NCORES):
        m = {
            "imx": np.ascontiguousarray(_pack_feature_major(x, core).astype(bf16)),
            "wh0": np.ascontiguousarray(wh0), "wh1": np.ascontiguousarray(wh1),
            "bia": bia,
        }
        in_maps.append(m)
    return in_maps


def unpack_outputs(results):
    out = np.empty((T, B, H), dtype=np.float32)
    cT = np.empty((B, H), dtype=np.float32)
    nT = np.empty((B, H), dtype=np.float32)
    for core in range(NCORES):
        r = results[core]
        o2h = r["out2h"].astype(np.float32)
        b0 = core * BS
        out[:, b0:b0 + SB, :] = o2h[:, 0:64, :].transpose(0, 2, 1)
        out[:, b0 + SB:b0 + BS, :] = o2h[:, 64:128, :].transpose(0, 2, 1)
        cf = r["cfin"].astype(np.float32)
        nf = r["nfin"].astype(np.float32)
        cT[b0:b0 + SB] = cf[0:64, :].T
        cT[b0 + SB:b0 + BS] = cf[64:128, :].T
        nT[b0:b0 + SB] = nf[0:64, :].T
        nT[b0 + SB:b0 + BS] = nf[64:128, :].T
    hT = out[T - 1]
    return out, hT[None], cT[None], nT[None]


def _install_ntff_hook():
    """Recreate the missing antenv.axon_hooks registry and wire the ctypes
    NTFF profile hook from trn_boot, so run_bass_kernel_spmd(trace=True)
    can capture a real neuron profile through axon."""
    import sys, types
    try:
        import antenv.axon_hooks  # noqa
        return True
    except ImportError:
        pass
    try:
        from trn_agent_boot.trn_boot import _ntff_profile_via_ctypes
        hook = _ntff_profile_via_ctypes("/opt/axon/libaxon_pjrt.so")
        if hook is None:
            return False
        mod = types.ModuleType("antenv.axon_hooks")
        mod._hook = hook
        mod.set_axon_ntff_profile_hook = lambda h: setattr(mod, "_hook", h)
        mod.get_axon_ntff_profile_hook = lambda: mod._hook
        sys.modules["antenv.axon_hooks"] = mod
        return True
    except Exception:
        return False


def kernel(obs_traj, img_embed_input, img_embed_input_merge, W_sp, b_sp, W_ih, W_hh, b_g):
    import os
    from concourse.bass_utils import run_bass_kernel_spmd

    in_maps = make_in_maps(obs_traj, img_embed_input, img_embed_input_merge,
                           W_sp, b_sp, W_ih, W_hh, b_g)
    trace = bool(int(os.environ.get("KERNEL_TRACE", "0")))
    if trace:
        trace = _install_ntff_hook()
    nc = _get_nc()
    res = run_bass_kernel_spmd(nc, in_maps, list(range(NCORES)), trace=trace,
                               tmpdir=os.environ.get("KERNEL_TRACE_DIR"))
    _CACHE["last_results"] = res
    return unpack_outputs(res.results)


# revision 19
# speedup vs baseline: 1.2907x; 1.2907x over previous
"""Trainium2 Bass kernel for nn_Encoder_68324339745355 (sLSTM encoder).

Math (per reference):
    x  = obs @ W_sp.T + b_sp + img1 + img2          # [T,B,E]
    g  = x @ W_ih.T + h @ W_hh.T + b_g              # per step, [B,4H]
    i=exp(it) f=sig(ft) z=tanh(zt) o=sig(ot)
    c = f*c + i*z ; n = f*n + i ; h = o*c/n
    outputs: (out [T,B,H], h[None], c[None], n[None])

Distribution: pure data-parallel, batch 16384 -> 8 cores x 2048.

Per-core layout ("stacked halves", feature-major):
  The 2048-batch shard splits into two 1024-halves. Every on-chip tensor is
  [128 partitions x 1024], partition = 64*half + feature. Feature-major means
  the recurrent matmul needs no transposes anywhere: PE contracts over
  partitions, gates come out stacked the same way, and ACT/DVE always run
  full 128-partition ops.

  XH0 [128,1024]: rows 0:64 = h (half0), rows 64:128 = x (half0)
  XH1 [128,1024]: rows 0:64 = x (half1), rows 64:128 = h (half1)
  (flip forced by partition-preservation of DVE ops: h emerges on the same
   partitions as its gate psum rows)

x = img1 + img2 + obs@W_sp.T is folded on the host during input packing (the
spatial-embedding GEMM is 0.1% of model FLOPs; folding it into the packed
input stream halves input DMA and removes all x-merge work on device), so x
enters each step's XH tiles by straight DMA.

Gates for (gate-chunk g, half): one K=128 matmul, lhsT = [W_hh.T ; W_ih.T]
stacked to match XH rows. The two halves write psum partitions 0:64 / 64:128
(different PE column groups) and are emitted adjacently so the two streams
run concurrently on the PE.

Sigmoid via tanh (only the exp_and_others ACT table set is ever loaded):
  f = 0.5*tanh(ft/2)+0.5 via a 4x-mode tensor_scalar, o likewise.
  1/n via the 1-instruction approximate-reciprocal custom DVE op (bf16-safe).

dtypes: bf16 on-chip except f32 PSUM accumulation and the f32 ACT bias.
"""

import numpy as np
import ml_dtypes

T, B, H, E, DIN = 32, 16384, 64, 64, 4
NCORES = 8
BS = B // NCORES          # 2048 batch per core
SB = BS // 2              # 1024 free dim (two halves stacked on partitions)
CH = 512                  # free-dim chunk (psum bank / moving-operand limit)
NCH = SB // CH

bf16 = ml_dtypes.bfloat16

_CACHE = {}


def _get_crf_op():
    """Register (once) a fused custom DVE op: out = Src1 * approx(1/Src0).

    Same bitwise-NOT seed as RECIPROCAL_APPROX_FAST but a single Newton
    step (max rel err ~0.4%, at bf16 noise level), freeing pipeline stages
    to fuse the multiply by c' — replaces reciprocal + tensor_tensor with
    one DVE instruction."""
    if "crf" in _CACHE:
        return _CACHE["crf"]
    import concourse.dve_ops as dve_ops
    from concourse.dve_spec import Spec, Src0, Src1, C0, C1, Bin, AluOp, lower
    from concourse.dve_uop import DveOpSpec
    import numpy as np_

    _not = Bin(AluOp.BITWISE_NOT, Src0, Src0)
    _y0 = _not * C0
    _y1 = _y0 * (C1 - Src0 * _y0)

    def _ref(in0, in1, s0, s1, imm2):
        not_x = (~in0.astype(np_.float32).view(np_.int32)).view(np_.float32)
        y0 = not_x * s0
        y1 = y0 * (s1 - in0 * y0)
        return (y1 * in1).astype(np_.float32)

    spec = Spec(body=_y1 * Src1, reference=_ref)
    name = "RECIP1_MUL_ANT"
    row = dve_ops._CUSTOM_DVE_ROW_BASE + len(dve_ops.OPS)
    assert row < 0x20
    shas = {}
    for ver in ("v3",):
        tmp = DveOpSpec(name=name, opcode=row, uops=lower(spec, ver=ver), rd1_en=True)
        shas[ver] = tmp.sha(ver)
    op = dve_ops.DveOp(name, spec, subdim=False, uops_sha=shas)
    dve_ops.OPS.append(op)
    dve_ops._SUB_OPCODE_FOR_NAME[name] = row
    _CACHE["crf"] = op
    return op


def _build(repeat=1):
    import concourse.bass as bass
    import concourse.bacc as bacc
    import concourse.tile as tile
    import concourse.mybir as mybir
    from concourse.dve_ops import RECIP_APPROX_FAST_CONSTS, RECIPROCAL_APPROX_FAST

    dt = mybir.dt
    AF = mybir.ActivationFunctionType
    ALU = mybir.AluOpType

    nc = bacc.Bacc("TRN2", target_bir_lowering=False, debug=False,
                   num_devices=NCORES)

    crf = _get_crf_op()
    imx = nc.dram_tensor("imx", [T, 128, SB], dt.bfloat16, kind="ExternalInput").ap()
    wh0 = nc.dram_tensor("wh0", [128, 256], dt.bfloat16, kind="ExternalInput").ap()
    wh1 = nc.dram_tensor("wh1", [128, 256], dt.bfloat16, kind="ExternalInput").ap()
    bia = nc.dram_tensor("bia", [128, 4], dt.float32, kind="ExternalInput").ap()
    out2h = nc.dram_tensor("out2h", [T, 128, SB], dt.bfloat16, kind="ExternalOutput").ap()
    cfin = nc.dram_tensor("cfin", [128, SB], dt.bfloat16, kind="ExternalOutput").ap()
    nfin = nc.dram_tensor("nfin", [128, SB], dt.bfloat16, kind="ExternalOutput").ap()

    rc = RECIP_APPROX_FAST_CONSTS

    with tile.TileContext(nc) as tc:
        with (
            tc.tile_pool(name="pw", bufs=1) as pw,
            tc.tile_pool(name="pxh", bufs=3) as pxh,
            tc.tile_pool(name="pst", bufs=2) as pst,
            tc.tile_pool(name="pg", bufs=2) as pg,
            tc.tile_pool(name="pt", bufs=2) as pt,
            tc.tile_pool(name="pps", bufs=2, space="PSUM") as pps,
        ):
            WH0 = pw.tile([128, 256], dt.bfloat16, name="WH0")
            WH1 = pw.tile([128, 256], dt.bfloat16, name="WH1")
            BIA = pw.tile([128, 4], dt.float32, name="BIA")
            nc.sync.dma_start(out=WH0[:], in_=wh0[:])
            nc.sync.dma_start(out=WH1[:], in_=wh1[:])
            nc.sync.dma_start(out=BIA[:], in_=bia[:])

            def new_xh(t):
                x0 = pxh.tile([128, SB], dt.bfloat16, name=f"XH0_{t}", tag="XH0")
                x1 = pxh.tile([128, SB], dt.bfloat16, name=f"XH1_{t}", tag="XH1")
                return x0, x1

            def load_x(t, x0, x1):
                # imx rows 0:64 = half1 x, rows 64:128 = half0 x
                nc.sync.dma_start(out=x0[64:128, :], in_=imx[t, 64:128, :])
                nc.sync.dma_start(out=x1[0:64, :], in_=imx[t, 0:64, :])

            XH0c, XH1c = new_xh(-1)
            nc.vector.memset(XH0c[0:64, :], 0.0)
            nc.vector.memset(XH1c[64:128, :], 0.0)
            load_x(0, XH0c, XH1c)
            Cc = pst.tile([128, SB], dt.bfloat16, name="C_init", tag="C")
            Nc = pst.tile([128, SB], dt.bfloat16, name="N_init", tag="N")
            nc.vector.memset(Cc[:], 0.0)
            nc.vector.memset(Nc[:], 0.0)

            for t in range(T * repeat):
                t = t % T
                XH0n, XH1n = new_xh(t)
                Cn = pst.tile([128, SB], dt.bfloat16, name=f"C_{t}", tag="C")
                Nn = pst.tile([128, SB], dt.bfloat16, name=f"N_{t}", tag="N")
                if t + 1 < T:
                    load_x(t + 1, XH0n, XH1n)

                for ch in range(NCH):
                    cs = slice(ch * CH, ch * CH + CH)
                    PS = [pps.tile([128, CH], dt.float32, name=f"PS{g}_{t}_{ch}",
                                   tag=f"PS{g}") for g in range(4)]
                    for g in range(4):
                        gsl = slice(64 * g, 64 * g + 64)
                        nc.tensor.matmul(PS[g][0:64, :], WH0[:, gsl], XH0c[:, cs],
                                         start=True, stop=True)
                        nc.tensor.matmul(PS[g][64:128, :], WH1[:, gsl], XH1c[:, cs],
                                         start=True, stop=True)

                    I = pg.tile([128, CH], dt.bfloat16, name=f"I_{t}_{ch}", tag="I")
                    Z = pg.tile([128, CH], dt.bfloat16, name=f"Z_{t}_{ch}", tag="Z")
                    U = pg.tile([128, CH], dt.bfloat16, name=f"U_{t}_{ch}", tag="U")
                    V = pg.tile([128, CH], dt.bfloat16, name=f"V_{t}_{ch}", tag="V")
                    nc.scalar.activation(I[:], PS[0][:], AF.Exp, bias=BIA[:, 0:1])
                    nc.scalar.activation(Z[:], PS[2][:], AF.Tanh, bias=BIA[:, 2:3])
                    nc.scalar.activation(U[:], PS[1][:], AF.Tanh, bias=BIA[:, 1:2], scale=0.5)
                    nc.scalar.activation(V[:], PS[3][:], AF.Tanh, bias=BIA[:, 3:4], scale=0.5)

                    F = pt.tile([128, CH], dt.bfloat16, name=f"F_{t}_{ch}", tag="F")
                    O = pt.tile([128, CH], dt.bfloat16, name=f"O_{t}_{ch}", tag="O")
                    IZ = pt.tile([128, CH], dt.bfloat16, name=f"IZ_{t}_{ch}", tag="IZ")
                    FC = pt.tile([128, CH], dt.bfloat16, name=f"FC_{t}_{ch}", tag="FC")
                    FN = pt.tile([128, CH], dt.bfloat16, name=f"FN_{t}_{ch}", tag="FN")
                    CR = pt.tile([128, CH], dt.bfloat16, name=f"CR_{t}_{ch}", tag="CR")

                    nc.vector.tensor_scalar(F[:], U[:], 0.5, 0.5, ALU.mult, ALU.add)
                    nc.vector.tensor_scalar(O[:], V[:], 0.5, 0.5, ALU.mult, ALU.add)
                    nc.vector.tensor_tensor(IZ[:], I[:], Z[:], ALU.mult)
                    nc.vector.tensor_tensor(FC[:], F[:], Cc[:, cs], ALU.mult)
                    nc.vector.tensor_tensor(Cn[:, cs], FC[:], IZ[:], ALU.add)
                    nc.vector.tensor_tensor(FN[:], F[:], Nc[:, cs], ALU.mult)
                    nc.vector.tensor_tensor(Nn[:, cs], FN[:], I[:], ALU.add)
                    # cr = c' * approx(1/n') in one fused DVE op
                    nc.vector._custom_dve(crf, out=CR[:], in0=Nn[:, cs], in1=Cn[:, cs],
                                          s0=rc["s0"], s1=rc["s1"])
                    nc.vector.tensor_tensor(XH0n[0:64, cs], O[0:64, :], CR[0:64, :], ALU.mult)
                    nc.vector.tensor_tensor(XH1n[64:128, cs], O[64:128, :], CR[64:128, :], ALU.mult)

                nc.sync.dma_start(out=out2h[t, 0:64, :], in_=XH0n[0:64, :])
                nc.sync.dma_start(out=out2h[t, 64:128, :], in_=XH1n[64:128, :])

                XH0c, XH1c = XH0n, XH1n
                Cc, Nc = Cn, Nn

            nc.sync.dma_start(out=cfin[:], in_=Cc[:])
            nc.sync.dma_start(out=nfin[:], in_=Nc[:])

    nc.compile()
    return nc


def _get_nc(repeat=1):
    key = f"nc{repeat}"
    if key not in _CACHE:
        _CACHE[key] = _build(repeat)
    return _CACHE[key]


def _pack_feature_major(arr, core):
    # arr [T, B, F] -> [T, 2F, SB] with rows 0:F = half1.T, rows F:2F = half0.T
    F = arr.shape[2]
    a = arr[:, core * BS:(core + 1) * BS, :].reshape(T, 2, SB, F)
    a = a.transpose(0, 1, 3, 2)  # [T, 2, F, SB]
    return np.concatenate([a[:, 1], a[:, 0]], axis=1)


def make_in_maps(obs_traj, img_embed_input, img_embed_input_merge, W_sp, b_sp,
                 W_ih, W_hh, b_g):
    obs_traj = np.asarray(obs_traj, dtype=np.float32)
    img1 = np.asarray(img_embed_input, dtype=np.float32)
    img2 = np.asarray(img_embed_input_merge, dtype=np.float32)
    W_sp = np.asarray(W_sp, dtype=np.float32)
    b_sp = np.asarray(b_sp, dtype=np.float32)
    W_ih = np.asarray(W_ih, dtype=np.float32)
    W_hh = np.asarray(W_hh, dtype=np.float32)
    b_g = np.asarray(b_g, dtype=np.float32)

    whh_t = W_hh.T.astype(bf16).astype(np.float32)   # [H, 4H]
    wih_t = W_ih.T.astype(bf16).astype(np.float32)   # [E, 4H]
    wh0 = np.concatenate([whh_t, wih_t], axis=0).astype(bf16)  # XH0 rows: [h ; x]
    wh1 = np.concatenate([wih_t, whh_t], axis=0).astype(bf16)  # XH1 rows: [x ; h]
    bias = b_g + W_ih @ b_sp
    bi = np.concatenate([bias[0:64], bias[0:64]])
    bf_ = np.concatenate([bias[64:128], bias[64:128]]) * 0.5
    bz = np.concatenate([bias[128:192], bias[128:192]])
    bo = np.concatenate([bias[192:256], bias[192:256]]) * 0.5
    bia = np.stack([bi, bf_, bz, bo], axis=1).astype(np.float32)

    # fold the spatial embedding into the packed x stream
    x = img1 + img2 + np.einsum("tbd,ed->tbe", obs_traj, W_sp)

    in_maps = []
    for core in range(NCORES):
        m = {
            "imx": np.ascontiguousarray(_pack_feature_major(x, core).astype(bf16)),
            "wh0": np.ascontiguousarray(wh0), "wh1": np.ascontiguousarray(wh1),
            "bia": bia,
        }
        in_maps.append(m)
    return in_maps


def unpack_outputs(results):
    out = np.empty((T, B, H), dtype=np.float32)
    cT = np.empty((B, H), dtype=np.float32)
    nT = np.empty((B, H), dtype=np.float32)
    for core in range(NCORES):
        r = results[core]
        o2h = r["out2h"].astype(np.float32)
        b0 = core * BS
        out[:, b0:b0 + SB, :] = o2h[:, 0:64, :].transpose(0, 2, 1)
        out[:, b0 + SB:b0 + BS, :] = o2h[:, 64:128, :].transpose(0, 2, 1)
        cf = r["cfin"].astype(np.float32)
        nf = r["nfin"].astype(np.float32)
        cT[b0:b0 + SB] = cf[0:64, :].T
        cT[b0 + SB:b0 + BS] = cf[64:128, :].T
        nT[b0:b0 + SB] = nf[0:64, :].T
        nT[b0 + SB:b0 + BS] = nf[64:128, :].T
    hT = out[T - 1]
    return out, hT[None], cT[None], nT[None]


def _install_ntff_hook():
    """Recreate the missing antenv.axon_hooks registry and wire the ctypes
    NTFF profile hook from trn_boot, so run_bass_kernel_spmd(trace=True)
    can capture a real neuron profile through axon."""
    import sys, types
    try:
        import antenv.axon_hooks  # noqa
        return True
    except ImportError:
        pass
    try:
        from trn_agent_boot.trn_boot import _ntff_profile_via_ctypes
        hook = _ntff_profile_via_ctypes("/opt/axon/libaxon_pjrt.so")
        if hook is None:
            return False
        mod = types.ModuleType("antenv.axon_hooks")
        mod._hook = hook
        mod.set_axon_ntff_profile_hook = lambda h: setattr(mod, "_hook", h)
        mod.get_axon_ntff_profile_hook = lambda: mod._hook
        sys.modules["antenv.axon_hooks"] = mod
        return True
    except Exception:
        return False


def kernel(obs_traj, img_embed_input, img_embed_input_merge, W_sp, b_sp, W_ih, W_hh, b_g):
    import os
    from concourse.bass_utils import run_bass_kernel_spmd

    in_maps = make_in_maps(obs_traj, img_embed_input, img_embed_input_merge,
                           W_sp, b_sp, W_ih, W_hh, b_g)
    trace = bool(int(os.environ.get("KERNEL_TRACE", "0")))
    if trace:
        trace = _install_ntff_hook()
    nc = _get_nc()
    res = run_bass_kernel_spmd(nc, in_maps, list(range(NCORES)), trace=trace,
                               tmpdir=os.environ.get("KERNEL_TRACE_DIR"))
    _CACHE["last_results"] = res
    return unpack_outputs(res.results)


# revision 28
# speedup vs baseline: 1.3662x; 1.0585x over previous
"""Trainium2 Bass kernel for nn_Encoder_68324339745355 (sLSTM encoder).

Math (per reference):
    x  = obs @ W_sp.T + b_sp + img1 + img2          # [T,B,E]
    g  = x @ W_ih.T + h @ W_hh.T + b_g              # per step, [B,4H]
    i=exp(it) f=sig(ft) z=tanh(zt) o=sig(ot)
    c = f*c + i*z ; n = f*n + i ; h = o*c/n
    outputs: (out [T,B,H], h[None], c[None], n[None])

Distribution: pure data-parallel, batch 16384 -> 8 cores x 2048.

Per-core layout ("stacked halves", feature-major):
  The 2048-batch shard splits into two 1024-halves. Every on-chip tensor is
  [128 partitions x 1024], partition = 64*half + feature. Feature-major means
  the recurrent matmul needs no transposes anywhere: PE contracts over
  partitions, gates come out stacked the same way, and ACT/DVE always run
  full 128-partition ops.

  XH0 [128,1024]: rows 0:64 = h (half0), rows 64:128 = x (half0)
  XH1 [128,1024]: rows 0:64 = x (half1), rows 64:128 = h (half1)
  (flip forced by partition-preservation of DVE ops: h emerges on the same
   partitions as its gate psum rows)

x = img1 + img2 + obs@W_sp.T is folded on the host during input packing (the
spatial-embedding GEMM is 0.1% of model FLOPs; folding it into the packed
input stream halves input DMA and removes all x-merge work on device), so x
enters each step's XH tiles by straight DMA.

Gates for (gate-chunk g, half): one K=128 matmul, lhsT = [W_hh.T ; W_ih.T]
stacked to match XH rows. The two halves write psum partitions 0:64 / 64:128
(different PE column groups) and are emitted adjacently so the two streams
run concurrently on the PE.

Sigmoid via tanh (only the exp_and_others ACT table set is ever loaded):
  f = 0.5*tanh(ft/2)+0.5 via a 4x-mode tensor_scalar, o likewise.
  1/n via the 1-instruction approximate-reciprocal custom DVE op (bf16-safe).

dtypes: bf16 on-chip except f32 PSUM accumulation and the f32 ACT bias.
"""

import numpy as np
import ml_dtypes

T, B, H, E, DIN = 32, 16384, 64, 64, 4
NCORES = 8
BS = B // NCORES          # 2048 batch per core
SB = BS // 2              # 1024 free dim (two halves stacked on partitions)
CH = 512                  # free-dim chunk (psum bank / moving-operand limit)
NCH = SB // CH

bf16 = ml_dtypes.bfloat16

_CACHE = {}


def _get_crf_op():
    """Register (once) a fused custom DVE op: out = Src1 * approx(1/Src0).

    Same bitwise-NOT seed as RECIPROCAL_APPROX_FAST but a single Newton
    step (max rel err ~0.4%, at bf16 noise level), freeing pipeline stages
    to fuse the multiply by c' — replaces reciprocal + tensor_tensor with
    one DVE instruction."""
    if "crf" in _CACHE:
        return _CACHE["crf"]
    import concourse.dve_ops as dve_ops
    from concourse.dve_spec import Spec, Src0, Src1, C0, C1, Bin, AluOp, lower
    from concourse.dve_uop import DveOpSpec
    import numpy as np_

    _not = Bin(AluOp.BITWISE_NOT, Src0, Src0)
    _y0 = _not * C0
    _y1 = _y0 * (C1 - Src0 * _y0)

    def _ref(in0, in1, s0, s1, imm2):
        not_x = (~in0.astype(np_.float32).view(np_.int32)).view(np_.float32)
        y0 = not_x * s0
        y1 = y0 * (s1 - in0 * y0)
        return (y1 * in1).astype(np_.float32)

    spec = Spec(body=_y1 * Src1, reference=_ref)
    name = "RECIP1_MUL_ANT"
    row = dve_ops._CUSTOM_DVE_ROW_BASE + len(dve_ops.OPS)
    assert row < 0x20
    shas = {}
    for ver in ("v3",):
        tmp = DveOpSpec(name=name, opcode=row, uops=lower(spec, ver=ver), rd1_en=True)
        shas[ver] = tmp.sha(ver)
    op = dve_ops.DveOp(name, spec, subdim=False, uops_sha=shas)
    dve_ops.OPS.append(op)
    dve_ops._SUB_OPCODE_FOR_NAME[name] = row
    _CACHE["crf"] = op
    return op


def _build(repeat=1):
    import concourse.bass as bass
    import concourse.bacc as bacc
    import concourse.tile as tile
    import concourse.mybir as mybir
    from concourse.dve_ops import RECIP_APPROX_FAST_CONSTS, RECIPROCAL_APPROX_FAST

    dt = mybir.dt
    AF = mybir.ActivationFunctionType
    ALU = mybir.AluOpType

    nc = bacc.Bacc("TRN2", target_bir_lowering=False, debug=False,
                   num_devices=NCORES)

    crf = _get_crf_op()
    imx = nc.dram_tensor("imx", [T, 128, SB], dt.bfloat16, kind="ExternalInput").ap()
    wh0 = nc.dram_tensor("wh0", [128, 256], dt.bfloat16, kind="ExternalInput").ap()
    wh1 = nc.dram_tensor("wh1", [128, 256], dt.bfloat16, kind="ExternalInput").ap()
    bia = nc.dram_tensor("bia", [128, 5], dt.float32, kind="ExternalInput").ap()
    out2h = nc.dram_tensor("out2h", [T, 128, SB], dt.bfloat16, kind="ExternalOutput").ap()
    cfin = nc.dram_tensor("cfin", [128, SB], dt.bfloat16, kind="ExternalOutput").ap()
    nfin = nc.dram_tensor("nfin", [128, SB], dt.bfloat16, kind="ExternalOutput").ap()

    rc = RECIP_APPROX_FAST_CONSTS

    with tile.TileContext(nc) as tc:
        with (
            tc.tile_pool(name="pw", bufs=1) as pw,
            tc.tile_pool(name="pxh", bufs=3) as pxh,
            tc.tile_pool(name="pst", bufs=2) as pst,
            tc.tile_pool(name="pg", bufs=2) as pg,
            tc.tile_pool(name="pt", bufs=2) as pt,
            tc.tile_pool(name="pps", bufs=2, space="PSUM") as pps,
        ):
            WH0 = pw.tile([128, 256], dt.bfloat16, name="WH0")
            WH1 = pw.tile([128, 256], dt.bfloat16, name="WH1")
            BIA = pw.tile([128, 5], dt.float32, name="BIA")
            nc.sync.dma_start(out=WH0[:], in_=wh0[:])
            nc.sync.dma_start(out=WH1[:], in_=wh1[:])
            nc.sync.dma_start(out=BIA[:], in_=bia[:])

            def new_xh(t):
                x0 = pxh.tile([128, SB], dt.bfloat16, name=f"XH0_{t}", tag="XH0")
                x1 = pxh.tile([128, SB], dt.bfloat16, name=f"XH1_{t}", tag="XH1")
                return x0, x1

            def load_x(t, x0, x1):
                # imx rows 0:64 = half1 x, rows 64:128 = half0 x
                nc.sync.dma_start(out=x0[64:128, :], in_=imx[t, 64:128, :])
                nc.sync.dma_start(out=x1[0:64, :], in_=imx[t, 0:64, :])

            XH0c, XH1c = new_xh(-1)
            nc.vector.memset(XH0c[0:64, :], 0.0)
            nc.vector.memset(XH1c[64:128, :], 0.0)
            load_x(0, XH0c, XH1c)
            Cc = pst.tile([128, SB], dt.bfloat16, name="C_init", tag="C")
            Nc = pst.tile([128, SB], dt.bfloat16, name="N_init", tag="N")
            nc.vector.memset(Cc[:], 0.0)
            nc.vector.memset(Nc[:], 0.0)

            for t in range(T * repeat):
                t = t % T
                XH0n, XH1n = new_xh(t)
                Cn = pst.tile([128, SB], dt.bfloat16, name=f"C_{t}", tag="C")
                Nn = pst.tile([128, SB], dt.bfloat16, name=f"N_{t}", tag="N")
                if t + 1 < T:
                    load_x(t + 1, XH0n, XH1n)

                for ch in range(NCH):
                    cs = slice(ch * CH, ch * CH + CH)
                    PS = [pps.tile([128, CH], dt.float32, name=f"PS{g}_{t}_{ch}",
                                   tag=f"PS{g}") for g in range(4)]
                    for g in range(4):
                        gsl = slice(64 * g, 64 * g + 64)
                        nc.tensor.matmul(PS[g][0:64, :], WH0[:, gsl], XH0c[:, cs],
                                         start=True, stop=True)
                        nc.tensor.matmul(PS[g][64:128, :], WH1[:, gsl], XH1c[:, cs],
                                         start=True, stop=True)

                    I = pg.tile([128, CH], dt.bfloat16, name=f"I_{t}_{ch}", tag="I")
                    Z = pg.tile([128, CH], dt.bfloat16, name=f"Z_{t}_{ch}", tag="Z")
                    U = pg.tile([128, CH], dt.bfloat16, name=f"U_{t}_{ch}", tag="U")
                    V = pg.tile([128, CH], dt.bfloat16, name=f"V_{t}_{ch}", tag="V")
                    nc.scalar.activation(I[:], PS[0][:], AF.Exp, bias=BIA[:, 0:1])
                    nc.scalar.activation(Z[:], PS[2][:], AF.Tanh, bias=BIA[:, 2:3])
                    nc.scalar.activation(U[:], PS[1][:], AF.Tanh, bias=BIA[:, 1:2], scale=0.5)
                    nc.scalar.activation(V[:], PS[3][:], AF.Tanh, bias=BIA[:, 3:4], scale=0.5)

                    F = pt.tile([128, CH], dt.bfloat16, name=f"F_{t}_{ch}", tag="F")
                    O = pt.tile([128, CH], dt.bfloat16, name=f"O_{t}_{ch}", tag="O")
                    IZ = pt.tile([128, CH], dt.bfloat16, name=f"IZ_{t}_{ch}", tag="IZ")
                    FC = pt.tile([128, CH], dt.bfloat16, name=f"FC_{t}_{ch}", tag="FC")
                    FN = pt.tile([128, CH], dt.bfloat16, name=f"FN_{t}_{ch}", tag="FN")
                    CR = pt.tile([128, CH], dt.bfloat16, name=f"CR_{t}_{ch}", tag="CR")

                    # emission order tracks ACT completion order (I, Z, U, V)
                    nc.vector.tensor_tensor(IZ[:], I[:], Z[:], ALU.mult)
                    nc.vector.tensor_scalar(F[:], U[:], 0.5, 0.5, ALU.mult, ALU.add)
                    nc.vector.tensor_tensor(FC[:], F[:], Cc[:, cs], ALU.mult)
                    nc.vector.tensor_tensor(FN[:], F[:], Nc[:, cs], ALU.mult)
                    nc.vector.tensor_tensor(Cn[:, cs], FC[:], IZ[:], ALU.add)
                    nc.vector.tensor_tensor(Nn[:, cs], FN[:], I[:], ALU.add)
                    # o = 0.5*V + 0.5 on ACT (Identity) — ACT has headroom, DVE is the bottleneck
                    nc.scalar.activation(O[:], V[:], AF.Identity, bias=BIA[:, 4:5], scale=0.5)
                    # cr = c' * approx(1/n') in one fused DVE op
                    nc.vector._custom_dve(crf, out=CR[:], in0=Nn[:, cs], in1=Cn[:, cs],
                                          s0=rc["s0"], s1=rc["s1"])
                    nc.vector.tensor_tensor(XH0n[0:64, cs], O[0:64, :], CR[0:64, :], ALU.mult)
                    nc.vector.tensor_tensor(XH1n[64:128, cs], O[64:128, :], CR[64:128, :], ALU.mult)

                nc.sync.dma_start(out=out2h[t, 0:64, :], in_=XH0n[0:64, :])
                nc.sync.dma_start(out=out2h[t, 64:128, :], in_=XH1n[64:128, :])

                XH0c, XH1c = XH0n, XH1n
                Cc, Nc = Cn, Nn

            nc.sync.dma_start(out=cfin[:], in_=Cc[:])
            nc.sync.dma_start(out=nfin[:], in_=Nc[:])

    nc.compile()
    return nc


def _get_nc(repeat=1):
    key = f"nc{repeat}"
    if key not in _CACHE:
        _CACHE[key] = _build(repeat)
    return _CACHE[key]


def _pack_feature_major(arr, core):
    # arr [T, B, F] -> [T, 2F, SB] with rows 0:F = half1.T, rows F:2F = half0.T
    F = arr.shape[2]
    a = arr[:, core * BS:(core + 1) * BS, :].reshape(T, 2, SB, F)
    a = a.transpose(0, 1, 3, 2)  # [T, 2, F, SB]
    return np.concatenate([a[:, 1], a[:, 0]], axis=1)


def make_in_maps(obs_traj, img_embed_input, img_embed_input_merge, W_sp, b_sp,
                 W_ih, W_hh, b_g):
    obs_traj = np.asarray(obs_traj, dtype=np.float32)
    img1 = np.asarray(img_embed_input, dtype=np.float32)
    img2 = np.asarray(img_embed_input_merge, dtype=np.float32)
    W_sp = np.asarray(W_sp, dtype=np.float32)
    b_sp = np.asarray(b_sp, dtype=np.float32)
    W_ih = np.asarray(W_ih, dtype=np.float32)
    W_hh = np.asarray(W_hh, dtype=np.float32)
    b_g = np.asarray(b_g, dtype=np.float32)

    whh_t = W_hh.T.astype(bf16).astype(np.float32)   # [H, 4H]
    wih_t = W_ih.T.astype(bf16).astype(np.float32)   # [E, 4H]
    wh0 = np.concatenate([whh_t, wih_t], axis=0).astype(bf16)  # XH0 rows: [h ; x]
    wh1 = np.concatenate([wih_t, whh_t], axis=0).astype(bf16)  # XH1 rows: [x ; h]
    bias = b_g + W_ih @ b_sp
    bi = np.concatenate([bias[0:64], bias[0:64]])
    bf_ = np.concatenate([bias[64:128], bias[64:128]]) * 0.5
    bz = np.concatenate([bias[128:192], bias[128:192]])
    bo = np.concatenate([bias[192:256], bias[192:256]]) * 0.5
    bia = np.stack([bi, bf_, bz, bo, np.full(128, 0.5, np.float32)], axis=1).astype(np.float32)

    # fold the spatial embedding into the packed x stream
    x = img1 + img2 + np.einsum("tbd,ed->tbe", obs_traj, W_sp)

    in_maps = []
    for core in range(NCORES):
        m = {
            "imx": np.ascontiguousarray(_pack_feature_major(x, core).astype(bf16)),
            "wh0": np.ascontiguousarray(wh0), "wh1": np.ascontiguousarray(wh1),
            "bia": bia,
        }
        in_maps.append(m)
    return in_maps


def unpack_outputs(results):
    out = np.empty((T, B, H), dtype=np.float32)
    cT = np.empty((B, H), dtype=np.float32)
    nT = np.empty((B, H), dtype=np.float32)
    for core in range(NCORES):
        r = results[core]
        o2h = r["out2h"].astype(np.float32)
        b0 = core * BS
        out[:, b0:b0 + SB, :] = o2h[:, 0:64, :].transpose(0, 2, 1)
        out[:, b0 + SB:b0 + BS, :] = o2h[:, 64:128, :].transpose(0, 2, 1)
        cf = r["cfin"].astype(np.float32)
        nf = r["nfin"].astype(np.float32)
        cT[b0:b0 + SB] = cf[0:64, :].T
        cT[b0 + SB:b0 + BS] = cf[64:128, :].T
        nT[b0:b0 + SB] = nf[0:64, :].T
        nT[b0 + SB:b0 + BS] = nf[64:128, :].T
    hT = out[T - 1]
    return out, hT[None], cT[None], nT[None]


def _install_ntff_hook():
    """Recreate the missing antenv.axon_hooks registry and wire the ctypes
    NTFF profile hook from trn_boot, so run_bass_kernel_spmd(trace=True)
    can capture a real neuron profile through axon."""
    import sys, types
    try:
        import antenv.axon_hooks  # noqa
        return True
    except ImportError:
        pass
    try:
        from trn_agent_boot.trn_boot import _ntff_profile_via_ctypes
        hook = _ntff_profile_via_ctypes("/opt/axon/libaxon_pjrt.so")
        if hook is None:
            return False
        mod = types.ModuleType("antenv.axon_hooks")
        mod._hook = hook
        mod.set_axon_ntff_profile_hook = lambda h: setattr(mod, "_hook", h)
        mod.get_axon_ntff_profile_hook = lambda: mod._hook
        sys.modules["antenv.axon_hooks"] = mod
        return True
    except Exception:
        return False


def kernel(obs_traj, img_embed_input, img_embed_input_merge, W_sp, b_sp, W_ih, W_hh, b_g):
    import os
    from concourse.bass_utils import run_bass_kernel_spmd

    in_maps = make_in_maps(obs_traj, img_embed_input, img_embed_input_merge,
                           W_sp, b_sp, W_ih, W_hh, b_g)
    trace = bool(int(os.environ.get("KERNEL_TRACE", "0")))
    if trace:
        trace = _install_ntff_hook()
    nc = _get_nc()
    res = run_bass_kernel_spmd(nc, in_maps, list(range(NCORES)), trace=trace,
                               tmpdir=os.environ.get("KERNEL_TRACE_DIR"))
    _CACHE["last_results"] = res
    return unpack_outputs(res.results)


# revision 30
# speedup vs baseline: 1.3690x; 1.0020x over previous
"""Trainium2 Bass kernel for nn_Encoder_68324339745355 (sLSTM encoder).

Math (per reference):
    x  = obs @ W_sp.T + b_sp + img1 + img2          # [T,B,E]
    g  = x @ W_ih.T + h @ W_hh.T + b_g              # per step, [B,4H]
    i=exp(it) f=sig(ft) z=tanh(zt) o=sig(ot)
    c = f*c + i*z ; n = f*n + i ; h = o*c/n
    outputs: (out [T,B,H], h[None], c[None], n[None])

Distribution: pure data-parallel, batch 16384 -> 8 cores x 2048.

Per-core layout ("stacked halves", feature-major):
  The 2048-batch shard splits into two 1024-halves. Every on-chip tensor is
  [128 partitions x 1024], partition = 64*half + feature. Feature-major means
  the recurrent matmul needs no transposes anywhere: PE contracts over
  partitions, gates come out stacked the same way, and ACT/DVE always run
  full 128-partition ops.

  XH0 [128,1024]: rows 0:64 = h (half0), rows 64:128 = x (half0)
  XH1 [128,1024]: rows 0:64 = x (half1), rows 64:128 = h (half1)
  (flip forced by partition-preservation of DVE ops: h emerges on the same
   partitions as its gate psum rows)

x = img1 + img2 + obs@W_sp.T is folded on the host during input packing (the
spatial-embedding GEMM is 0.1% of model FLOPs; folding it into the packed
input stream halves input DMA and removes all x-merge work on device), so x
enters each step's XH tiles by straight DMA.

Gates for (gate-chunk g, half): one K=128 matmul, lhsT = [W_hh.T ; W_ih.T]
stacked to match XH rows. The two halves write psum partitions 0:64 / 64:128
(different PE column groups) and are emitted adjacently so the two streams
run concurrently on the PE.

Sigmoid via tanh (only the exp_and_others ACT table set is ever loaded):
  f = 0.5*tanh(ft/2)+0.5 via a 4x-mode tensor_scalar, o likewise.
  1/n via the 1-instruction approximate-reciprocal custom DVE op (bf16-safe).

dtypes: bf16 on-chip except f32 PSUM accumulation and the f32 ACT bias.
"""

import numpy as np
import ml_dtypes

T, B, H, E, DIN = 32, 16384, 64, 64, 4
NCORES = 8
BS = B // NCORES          # 2048 batch per core
SB = BS // 2              # 1024 free dim (two halves stacked on partitions)
CH = 512                  # free-dim chunk (psum bank / moving-operand limit)
NCH = SB // CH

bf16 = ml_dtypes.bfloat16

_CACHE = {}


def _get_crf_op():
    """Register (once) a fused custom DVE op: out = Src1 * approx(1/Src0).

    Same bitwise-NOT seed as RECIPROCAL_APPROX_FAST but a single Newton
    step (max rel err ~0.4%, at bf16 noise level), freeing pipeline stages
    to fuse the multiply by c' — replaces reciprocal + tensor_tensor with
    one DVE instruction."""
    if "crf" in _CACHE:
        return _CACHE["crf"]
    import concourse.dve_ops as dve_ops
    from concourse.dve_spec import Spec, Src0, Src1, C0, C1, Bin, AluOp, lower
    from concourse.dve_uop import DveOpSpec
    import numpy as np_

    _not = Bin(AluOp.BITWISE_NOT, Src0, Src0)
    _y0 = _not * C0
    _y1 = _y0 * (C1 - Src0 * _y0)

    def _ref(in0, in1, s0, s1, imm2):
        not_x = (~in0.astype(np_.float32).view(np_.int32)).view(np_.float32)
        y0 = not_x * s0
        y1 = y0 * (s1 - in0 * y0)
        return (y1 * in1).astype(np_.float32)

    spec = Spec(body=_y1 * Src1, reference=_ref)
    name = "RECIP1_MUL_ANT"
    row = dve_ops._CUSTOM_DVE_ROW_BASE + len(dve_ops.OPS)
    assert row < 0x20
    shas = {}
    for ver in ("v3",):
        tmp = DveOpSpec(name=name, opcode=row, uops=lower(spec, ver=ver), rd1_en=True)
        shas[ver] = tmp.sha(ver)
    op = dve_ops.DveOp(name, spec, subdim=False, uops_sha=shas)
    dve_ops.OPS.append(op)
    dve_ops._SUB_OPCODE_FOR_NAME[name] = row
    _CACHE["crf"] = op
    return op


def _build(repeat=1):
    import concourse.bass as bass
    import concourse.bacc as bacc
    import concourse.tile as tile
    import concourse.mybir as mybir
    from concourse.dve_ops import RECIP_APPROX_FAST_CONSTS, RECIPROCAL_APPROX_FAST

    dt = mybir.dt
    AF = mybir.ActivationFunctionType
    ALU = mybir.AluOpType

    nc = bacc.Bacc("TRN2", target_bir_lowering=False, debug=False,
                   num_devices=NCORES)

    crf = _get_crf_op()
    imx = nc.dram_tensor("imx", [T, 128, SB], dt.bfloat16, kind="ExternalInput").ap()
    wh0 = nc.dram_tensor("wh0", [128, 256], dt.bfloat16, kind="ExternalInput").ap()
    wh1 = nc.dram_tensor("wh1", [128, 256], dt.bfloat16, kind="ExternalInput").ap()
    bia = nc.dram_tensor("bia", [128, 5], dt.float32, kind="ExternalInput").ap()
    out2h = nc.dram_tensor("out2h", [T, 128, SB], dt.bfloat16, kind="ExternalOutput").ap()
    cfin = nc.dram_tensor("cfin", [128, SB], dt.bfloat16, kind="ExternalOutput").ap()
    nfin = nc.dram_tensor("nfin", [128, SB], dt.bfloat16, kind="ExternalOutput").ap()

    rc = RECIP_APPROX_FAST_CONSTS

    with tile.TileContext(nc) as tc:
        with (
            tc.tile_pool(name="pw", bufs=1) as pw,
            tc.tile_pool(name="pxh", bufs=3) as pxh,
            tc.tile_pool(name="pst", bufs=2) as pst,
            tc.tile_pool(name="pg", bufs=2) as pg,
            tc.tile_pool(name="pt", bufs=2) as pt,
            tc.tile_pool(name="pps", bufs=2, space="PSUM") as pps,
        ):
            WH0 = pw.tile([128, 256], dt.bfloat16, name="WH0")
            WH1 = pw.tile([128, 256], dt.bfloat16, name="WH1")
            BIA = pw.tile([128, 5], dt.float32, name="BIA")
            nc.sync.dma_start(out=WH0[:], in_=wh0[:])
            nc.sync.dma_start(out=WH1[:], in_=wh1[:])
            nc.sync.dma_start(out=BIA[:], in_=bia[:])

            def new_xh(t):
                x0 = pxh.tile([128, SB], dt.bfloat16, name=f"XH0_{t}", tag="XH0")
                x1 = pxh.tile([128, SB], dt.bfloat16, name=f"XH1_{t}", tag="XH1")
                return x0, x1

            def load_x(t, x0, x1):
                # imx rows 0:64 = half1 x, rows 64:128 = half0 x
                nc.sync.dma_start(out=x0[64:128, :], in_=imx[t, 64:128, :])
                nc.sync.dma_start(out=x1[0:64, :], in_=imx[t, 0:64, :])

            XH0c, XH1c = new_xh(-1)
            nc.vector.memset(XH0c[0:64, :], 0.0)
            nc.vector.memset(XH1c[64:128, :], 0.0)
            load_x(0, XH0c, XH1c)
            Cc = pst.tile([128, SB], dt.bfloat16, name="C_init", tag="C")
            Nc = pst.tile([128, SB], dt.bfloat16, name="N_init", tag="N")
            nc.vector.memset(Cc[:], 0.0)
            nc.vector.memset(Nc[:], 0.0)

            for t in range(T * repeat):
                t = t % T
                XH0n, XH1n = new_xh(t)
                Cn = pst.tile([128, SB], dt.bfloat16, name=f"C_{t}", tag="C")
                Nn = pst.tile([128, SB], dt.bfloat16, name=f"N_{t}", tag="N")
                if t + 1 < T:
                    load_x(t + 1, XH0n, XH1n)

                for ch in range(NCH):
                    cs = slice(ch * CH, ch * CH + CH)
                    PS = [pps.tile([128, CH], dt.float32, name=f"PS{g}_{t}_{ch}",
                                   tag=f"PS{g}") for g in range(4)]
                    for g in range(4):
                        gsl = slice(64 * g, 64 * g + 64)
                        nc.tensor.matmul(PS[g][0:64, :], WH0[:, gsl], XH0c[:, cs],
                                         start=True, stop=True)
                        nc.tensor.matmul(PS[g][64:128, :], WH1[:, gsl], XH1c[:, cs],
                                         start=True, stop=True)

                    I = pg.tile([128, CH], dt.bfloat16, name=f"I_{t}_{ch}", tag="I")
                    Z = pg.tile([128, CH], dt.bfloat16, name=f"Z_{t}_{ch}", tag="Z")
                    U = pg.tile([128, CH], dt.bfloat16, name=f"U_{t}_{ch}", tag="U")
                    V = pg.tile([128, CH], dt.bfloat16, name=f"V_{t}_{ch}", tag="V")
                    nc.scalar.activation(I[:], PS[0][:], AF.Exp, bias=BIA[:, 0:1])
                    nc.scalar.activation(Z[:], PS[2][:], AF.Tanh, bias=BIA[:, 2:3])
                    nc.scalar.activation(U[:], PS[1][:], AF.Tanh, bias=BIA[:, 1:2], scale=0.5)
                    nc.scalar.activation(V[:], PS[3][:], AF.Tanh, bias=BIA[:, 3:4], scale=0.5)

                    F = pt.tile([128, CH], dt.bfloat16, name=f"F_{t}_{ch}", tag="F")
                    O = pt.tile([128, CH], dt.bfloat16, name=f"O_{t}_{ch}", tag="O")
                    IZ = pt.tile([128, CH], dt.bfloat16, name=f"IZ_{t}_{ch}", tag="IZ")
                    FC = pt.tile([128, CH], dt.bfloat16, name=f"FC_{t}_{ch}", tag="FC")
                    FN = pt.tile([128, CH], dt.bfloat16, name=f"FN_{t}_{ch}", tag="FN")
                    CR = pt.tile([128, CH], dt.bfloat16, name=f"CR_{t}_{ch}", tag="CR")

                    # emission order tracks ACT completion order (I, Z, U, V)
                    nc.vector.tensor_tensor(IZ[:], I[:], Z[:], ALU.mult)
                    nc.vector.tensor_scalar(F[:], U[:], 0.5, 0.5, ALU.mult, ALU.add)
                    nc.vector.tensor_tensor(FC[:], F[:], Cc[:, cs], ALU.mult)
                    nc.vector.tensor_tensor(FN[:], F[:], Nc[:, cs], ALU.mult)
                    nc.vector.tensor_tensor(Cn[:, cs], FC[:], IZ[:], ALU.add)
                    nc.vector.tensor_tensor(Nn[:, cs], FN[:], I[:], ALU.add)
                    # o = 0.5*V + 0.5 on ACT (Identity) — ACT has headroom, DVE is the bottleneck
                    nc.scalar.activation(O[:], V[:], AF.Identity, bias=BIA[:, 4:5], scale=0.5)
                    # cr = c' * approx(1/n') in one fused DVE op
                    nc.vector._custom_dve(crf, out=CR[:], in0=Nn[:, cs], in1=Cn[:, cs],
                                          s0=rc["s0"], s1=rc["s1"])
                    nc.vector.tensor_tensor(XH0n[0:64, cs], O[0:64, :], CR[0:64, :], ALU.mult)
                    nc.vector.tensor_tensor(XH1n[64:128, cs], O[64:128, :], CR[64:128, :], ALU.mult)

                nc.sync.dma_start(out=out2h[t, 0:64, :], in_=XH0n[0:64, :])
                nc.sync.dma_start(out=out2h[t, 64:128, :], in_=XH1n[64:128, :])

                XH0c, XH1c = XH0n, XH1n
                Cc, Nc = Cn, Nn

            nc.sync.dma_start(out=cfin[:], in_=Cc[:])
            nc.sync.dma_start(out=nfin[:], in_=Nc[:])

    nc.compile()
    return nc


def _get_nc(repeat=1):
    key = f"nc{repeat}"
    if key not in _CACHE:
        _CACHE[key] = _build(repeat)
    return _CACHE[key]


def _pack_feature_major(arr, core):
    # arr [T, B, F] -> [T, 2F, SB] with rows 0:F = half1.T, rows F:2F = half0.T
    F = arr.shape[2]
    a = arr[:, core * BS:(core + 1) * BS, :].reshape(T, 2, SB, F)
    a = a.transpose(0, 1, 3, 2)  # [T, 2, F, SB]
    return np.concatenate([a[:, 1], a[:, 0]], axis=1)


def make_in_maps(obs_traj, img_embed_input, img_embed_input_merge, W_sp, b_sp,
                 W_ih, W_hh, b_g):
    obs_traj = np.asarray(obs_traj, dtype=np.float32)
    img1 = np.asarray(img_embed_input, dtype=np.float32)
    img2 = np.asarray(img_embed_input_merge, dtype=np.float32)
    W_sp = np.asarray(W_sp, dtype=np.float32)
    b_sp = np.asarray(b_sp, dtype=np.float32)
    W_ih = np.asarray(W_ih, dtype=np.float32)
    W_hh = np.asarray(W_hh, dtype=np.float32)
    b_g = np.asarray(b_g, dtype=np.float32)

    whh_t = W_hh.T.astype(bf16).astype(np.float32)   # [H, 4H]
    wih_t = W_ih.T.astype(bf16).astype(np.float32)   # [E, 4H]
    wh0 = np.concatenate([whh_t, wih_t], axis=0).astype(bf16)  # XH0 rows: [h ; x]
    wh1 = np.concatenate([wih_t, whh_t], axis=0).astype(bf16)  # XH1 rows: [x ; h]
    bias = b_g + W_ih @ b_sp
    bi = np.concatenate([bias[0:64], bias[0:64]])
    bf_ = np.concatenate([bias[64:128], bias[64:128]]) * 0.5
    bz = np.concatenate([bias[128:192], bias[128:192]])
    bo = np.concatenate([bias[192:256], bias[192:256]]) * 0.5
    bia = np.stack([bi, bf_, bz, bo, np.full(128, 0.5, np.float32)], axis=1).astype(np.float32)

    # fold the spatial embedding into the packed x stream
    x = img1 + img2 + np.einsum("tbd,ed->tbe", obs_traj, W_sp)

    in_maps = []
    for core in range(NCORES):
        m = {
            "imx": np.ascontiguousarray(_pack_feature_major(x, core).astype(bf16)),
            "wh0": np.ascontiguousarray(wh0), "wh1": np.ascontiguousarray(wh1),
            "bia": bia,
        }
        in_maps.append(m)
    return in_maps


def unpack_outputs(results):
    out = np.empty((T, B, H), dtype=np.float32)
    cT = np.empty((B, H), dtype=np.float32)
    nT = np.empty((B, H), dtype=np.float32)
    for core in range(NCORES):
        r = results[core]
        o2h = r["out2h"].astype(np.float32)
        b0 = core * BS
        out[:, b0:b0 + SB, :] = o2h[:, 0:64, :].transpose(0, 2, 1)
        out[:, b0 + SB:b0 + BS, :] = o2h[:, 64:128, :].transpose(0, 2, 1)
        cf = r["cfin"].astype(np.float32)
        nf = r["nfin"].astype(np.float32)
        cT[b0:b0 + SB] = cf[0:64, :].T
        cT[b0 + SB:b0 + BS] = cf[64:128, :].T
        nT[b0:b0 + SB] = nf[0:64, :].T
        nT[b0 + SB:b0 + BS] = nf[64:128, :].T
    hT = out[T - 1]
    return out, hT[None], cT[None], nT[None]


def _install_ntff_hook():
    """Recreate the missing antenv.axon_hooks registry and wire the ctypes
    NTFF profile hook from trn_boot, so run_bass_kernel_spmd(trace=True)
    can capture a real neuron profile through axon."""
    import sys, types
    try:
        import antenv.axon_hooks  # noqa
        return True
    except ImportError:
        pass
    try:
        from trn_agent_boot.trn_boot import _ntff_profile_via_ctypes
        hook = _ntff_profile_via_ctypes("/opt/axon/libaxon_pjrt.so")
        if hook is None:
            return False
        mod = types.ModuleType("antenv.axon_hooks")
        mod._hook = hook
        mod.set_axon_ntff_profile_hook = lambda h: setattr(mod, "_hook", h)
        mod.get_axon_ntff_profile_hook = lambda: mod._hook
        sys.modules["antenv.axon_hooks"] = mod
        return True
    except Exception:
        return False


def kernel(obs_traj, img_embed_input, img_embed_input_merge, W_sp, b_sp, W_ih, W_hh, b_g):
    import os
    from concourse.bass_utils import run_bass_kernel_spmd

    in_maps = make_in_maps(obs_traj, img_embed_input, img_embed_input_merge,
                           W_sp, b_sp, W_ih, W_hh, b_g)
    trace = bool(int(os.environ.get("KERNEL_TRACE", "0")))
    if trace:
        trace = _install_ntff_hook()
    nc = _get_nc()
    res = run_bass_kernel_spmd(nc, in_maps, list(range(NCORES)), trace=trace,
                               tmpdir=os.environ.get("KERNEL_TRACE_DIR"))
    _CACHE["last_results"] = res
    return unpack_outputs(res.results)
